# revision 13
# baseline (speedup 1.0000x reference)
"""Trainium2 Bass kernel for nn_CategoricalLayer (segment gather + soft-evidence log).

Math (per node n, batch b):
    out[n, b] = log( q * a + (1 - a) ) = log1p( (q - 1) * a )
      where q = missing[v,b] ? 1.0 : clamp(params[psids[n] + data[v,b]], 1e-10)
            v = vids[n], a = alphas[v,b]
(params = exp(rand * -4) >= e^-4 >> 1e-10, so the clamp is a no-op.)

Strategy (8 NeuronCores, variable-sharded: 32 vars = 512 nodes per core,
full 4096-sample batch per core):
  - Host (layout only): per-node lookup tables T[n, c] = params[psids[n]+c] - 1
    for c < 256, and 0.0 for c in [256, 512) (missing sentinel -> exact 0).
    Combined index idx = data + 256*missing precomputed as int16 in the
    16-partition wrapped layout ap_gather consumes.
  - Device: ap_gather on GPSIMD (SBUF->SBUF, one gpsimd core per variable:
    its 16 channels = the variable's 16 nodes, shared index stream = that
    variable's data row). Alphas are broadcast across each variable's 16
    node-partitions by a tiny PE matmul with a 0/1 selector (f32r, exact).
    DVE computes t = (q-1)*a; ACT computes Ln(t + 1) = log1p(t) straight to
    fp16; results stream to DRAM.
  - Host: reshape per-core [4, 128, 4096] fp16 -> [512, 4096] f32 rows
    (node order is naturally contiguous; no unscrambling).
"""
import sys
import os

for _p in ("/opt/trn_rl_repo",):
    if _p not in sys.path and os.path.isdir(_p):
        sys.path.insert(0, _p)

import numpy as np

import concourse.bass as bass
import concourse.bacc as bacc
import concourse.tile as tile
from concourse import mybir
from concourse.bass import AP
from concourse.bass_utils import run_bass_kernel_spmd

V = 256          # num variables
C = 256          # categories
B = 4096         # batch
NUM_NODES = 4096
NCORES = 8
J = 16           # nodes per variable
VPC = V // NCORES            # 32 variables per core
NPC = VPC * J                # 512 nodes per core
NG = NPC // 128              # 4 groups of 128 nodes (8 vars each)
GV = 128 // J                # 8 variables per group
NE = 2 * C                   # table entries per node (256 cats + 256 sentinel)
CHUNK = 2048                 # batch columns per pipeline iteration
NCHUNK = B // CHUNK          # 2
MM = 512                     # matmul free size (one PSUM bank of fp32)

TRACE = False            # set True (e.g. from test.py) to capture a profile
LAST_RESULT = {}         # exec_time_ns etc. stashed here when TRACE

_MAXW = 1  # this toolchain's walrus encodes at most one sync wait per instruction


def _legalize_waits(nc):
    """Split multi-wait instructions into single-wait NoOp prefixes."""
    for _name, bb in nc.bb_map.items():
        insts = bb.bb.instructions
        new = []
        changed = False
        for ins in insts:
            si = ins.sync_info
            if si is not None and si.on_wait and len(si.on_wait) > _MAXW:
                waits = list(si.on_wait)
                extra, keep = waits[:-_MAXW], waits[-_MAXW:]
                for i, w in enumerate(extra):
                    nop = mybir.InstNoOp(name=f"{ins.name}-sw{i}", ins=[], outs=[])
                    nop.engine = ins.engine
                    nop.sync_info = mybir.SyncInfo(on_wait=[w], on_update=[])
                    new.append(nop)
                ins.sync_info = mybir.SyncInfo(
                    on_wait=keep, on_update=list(si.on_update or [])
                )
                changed = True
            new.append(ins)
        if changed:
            bb.bb.instructions = new


def _build_program():
    nc = bacc.Bacc(
        "TRN2",
        target_bir_lowering=False,
        debug=False,
        num_devices=NCORES,
    )

    tbl = nc.dram_tensor("tbl", [NPC, NE], mybir.dt.float32, kind="ExternalInput")
    d16 = nc.dram_tensor("d16", [128, NG * (B // J)], mybir.dt.int16,
                         kind="ExternalInput")
    alf = nc.dram_tensor("alf", [GV, NG * B], mybir.dt.float32r,
                         kind="ExternalInput")
    wsl = nc.dram_tensor("wsl", [GV, 128], mybir.dt.float32r,
                         kind="ExternalInput")
    out = nc.dram_tensor("out", [NG, 128, B], mybir.dt.float16,
                         kind="ExternalOutput")

    from contextlib import ExitStack

    with tile.TileContext(nc) as tc, ExitStack() as ctx:
        const_pool = ctx.enter_context(tc.tile_pool(name="const", bufs=1))
        g_pool = ctx.enter_context(tc.tile_pool(name="g", bufs=4))
        t_pool = ctx.enter_context(tc.tile_pool(name="t", bufs=4))
        o_pool = ctx.enter_context(tc.tile_pool(name="o", bufs=8))
        ps_pool = ctx.enter_context(tc.tile_pool(name="ps", bufs=2, space="PSUM"))

        t_s = [const_pool.tile([128, NE], mybir.dt.float32, name=f"tsg{g}")
               for g in range(NG)]
        d_s = const_pool.tile([128, NG * (B // J)], mybir.dt.int16)
        a_s = const_pool.tile([GV, NG * B], mybir.dt.float32r)
        w_s = const_pool.tile([GV, 128], mybir.dt.float32r)

        # The first gather needs only t0 + group-0's idx columns; everything
        # else (wsl/alf for the matmuls, t1..t3, remaining idx) can trail.
        # Issue in that order on one engine so the HWDGE/DMA queues match it.
        iw0 = B // J
        nc.sync.dma_start(out=t_s[0][:], in_=tbl[0:128, :])
        nc.sync.dma_start(out=w_s[:], in_=wsl[:])
        nc.sync.dma_start(out=a_s[:, 0:CHUNK], in_=alf[:, 0:CHUNK])
        nc.sync.dma_start(out=d_s[:, 0:iw0], in_=d16[:, 0:iw0])
        nc.sync.dma_start(out=a_s[:, CHUNK:NG * B], in_=alf[:, CHUNK:NG * B])
        nc.sync.dma_start(out=d_s[:, iw0:NG * iw0], in_=d16[:, iw0:NG * iw0])
        for g in range(1, NG):
            nc.sync.dma_start(out=t_s[g][:], in_=tbl[g * 128:(g + 1) * 128, :])

        # Warm the ACT Ln table (1283ns load) off the critical path.
        warm = const_pool.tile([GV, 128], mybir.dt.float32)
        nc.scalar.activation(
            out=warm[:], in_=w_s[:],
            func=mybir.ActivationFunctionType.Ln, bias=1.0)

        # (group, col offset, cols): full chunks, with the tail tapered so the
        # final DVE->ACT->DMA drain after the last gather is short.
        sched = []
        for g in range(NG - 1):
            for h in range(NCHUNK):
                sched.append((g, h * CHUNK, CHUNK))
        c0 = 0
        for cl in (1024, 1024, 1024, 512, 256, 256):
            sched.append((NG - 1, c0, cl))
            c0 += cl

        iw = B // J          # idx free columns per group (256)
        for g, c0, cl in sched:
            f0 = g * iw + c0 // J              # idx tile free offset
            G = g_pool.tile([128, cl], mybir.dt.float32, tag="G")
            nc.gpsimd.ap_gather(
                out_ap=G[:], in_ap=t_s[g][:],
                idxs_ap=d_s[:, f0:f0 + cl // J],
                channels=128, num_elems=NE, d=1, num_idxs=cl)

            A = ps_pool.tile([128, cl], mybir.dt.float32, tag="A")
            for m in range(-(-cl // MM)):
                m0, m1 = m * MM, min((m + 1) * MM, cl)
                nc.tensor.matmul(
                    out=A[:, m0:m1],
                    lhsT=w_s[:],
                    rhs=a_s[:, g * B + c0 + m0:g * B + c0 + m1],
                    start=True, stop=True)

            T = t_pool.tile([128, cl], mybir.dt.float32, tag="T")
            nc.vector.tensor_tensor(
                out=T[:], in0=G[:], in1=A[:], op=mybir.AluOpType.mult)

            O = o_pool.tile([128, cl], mybir.dt.float16, tag="O")
            nc.scalar.activation(
                out=O[:], in_=T[:],
                func=mybir.ActivationFunctionType.Ln, bias=1.0)
            nc.scalar.dma_start(out=out[g, :, c0:c0 + cl], in_=O[:])

    nc.compile()
    _legalize_waits(nc)
    return nc


_prog_cache = {}


def _get_program(key=None):
    if "p" not in _prog_cache:
        _prog_cache["p"] = _build_program()
    return _prog_cache["p"]


def kernel(data, vids, psids, params, missing_mask, alphas):
    data = np.asarray(data).astype(np.int32, copy=False)
    vids = np.asarray(vids).astype(np.int64, copy=False)
    psids = np.asarray(psids).astype(np.int64, copy=False)
    params = np.asarray(params).astype(np.float32, copy=False)
    missing = np.asarray(missing_mask).astype(bool, copy=False)
    alphas = np.asarray(alphas).astype(np.float32, copy=False)

    num_nodes = vids.shape[0]
    assert num_nodes == NUM_NODES and data.shape == (V, B)

    # node -> variable map; each variable must own J consecutive nodes
    vb = vids.reshape(NUM_NODES // J, J)
    assert (vb == vb[:, :1]).all(), "nodes of a 16-block must share a variable"
    blk_vid = vb[:, 0]                                   # [256] variable per block

    # full per-node category tables, storing q - 1
    gi = psids[:, None] + np.arange(C, dtype=np.int64)[None, :]
    tfull = params[gi].astype(np.float32) - np.float32(1.0)   # [4096, 256]

    # combined gather index (data + 256*missing), int16
    dcomb = (data + (missing.astype(np.int32) << 8)).astype(np.int16)  # [V, B]

    in_maps = []
    for ci in range(NCORES):
        n0 = ci * NPC
        vlist = blk_vid[ci * VPC:(ci + 1) * VPC]          # [32] vars of this core

        tblh = np.zeros((NPC, NE), dtype=np.float32)
        tblh[:, :C] = tfull[n0:n0 + NPC]

        # wrapped idx layout: partition 16j+s, free 256g+f  <-  dc[8g+j, 16f+s]
        dc = dcomb[vlist]                                 # [32, B]
        d16h = (dc.reshape(NG, GV, B // J, J)
                  .transpose(1, 3, 0, 2)
                  .reshape(128, NG * (B // J)))

        # alphas: row j, col g*B+b  <-  alphas[vlist[8g+j], b]
        alh = (alphas[vlist].reshape(NG, GV, B)
                            .transpose(1, 0, 2)
                            .reshape(GV, NG * B))

        wsh = np.repeat(np.eye(GV, dtype=np.float32), J, axis=1)  # [8, 128]

        in_maps.append(dict(
            tbl=tblh,
            d16=np.ascontiguousarray(d16h),
            alf=np.ascontiguousarray(alh),
            wsl=wsh,
        ))

    nc = _get_program()
    res = run_bass_kernel_spmd(nc, in_maps, list(range(NCORES)), trace=TRACE)
    if TRACE:
        LAST_RESULT["exec_time_ns"] = getattr(res, "exec_time_ns", None)
        LAST_RESULT["mean_exec_time_ns"] = getattr(res, "mean_exec_time_ns", None)
        LAST_RESULT["profile_json"] = getattr(res, "profile_json", None)

    out_full = np.empty((NUM_NODES, B), dtype=np.float32)
    for ci in range(NCORES):
        o = np.asarray(res.results[ci]["out"])            # [4, 128, 4096] fp16
        out_full[ci * NPC:(ci + 1) * NPC] = o.reshape(NPC, B).astype(np.float32)
    return out_full


# revision 15
# speedup vs baseline: 1.1087x; 1.1087x over previous
"""Trainium2 Bass kernel for nn_CategoricalLayer (segment gather + soft-evidence log).

Math (per node n, batch b):
    out[n, b] = log( q * a + (1 - a) ) = log1p( (q - 1) * a )
      where q = missing[v,b] ? 1.0 : clamp(params[psids[n] + data[v,b]], 1e-10)
            v = vids[n], a = alphas[v,b]
(params = exp(rand * -4) >= e^-4 >> 1e-10, so the clamp is a no-op.)

Strategy (8 NeuronCores, variable-sharded: 32 vars = 512 nodes per core,
full 4096-sample batch per core):
  - Host (layout only): per-node lookup tables T[n, c] = params[psids[n]+c] - 1
    for c < 256, and 0.0 for c in [256, 512) (missing sentinel -> exact 0).
    Combined index idx = data + 256*missing precomputed as int16 in the
    16-partition wrapped layout ap_gather consumes.
  - Device: ap_gather on GPSIMD (SBUF->SBUF, one gpsimd core per variable:
    its 16 channels = the variable's 16 nodes, shared index stream = that
    variable's data row). Alphas are broadcast across each variable's 16
    node-partitions by a tiny PE matmul with a 0/1 selector (f32r, exact).
    DVE computes t = (q-1)*a; ACT computes Ln(t + 1) = log1p(t) straight to
    fp16; results stream to DRAM.
  - Host: reshape per-core [4, 128, 4096] fp16 -> [512, 4096] f32 rows
    (node order is naturally contiguous; no unscrambling).
"""
import sys
import os

for _p in ("/opt/trn_rl_repo",):
    if _p not in sys.path and os.path.isdir(_p):
        sys.path.insert(0, _p)

import numpy as np

import concourse.bass as bass
import concourse.bacc as bacc
import concourse.tile as tile
from concourse import mybir
from concourse.bass import AP
from concourse.bass_utils import run_bass_kernel_spmd

V = 256          # num variables
C = 256          # categories
B = 4096         # batch
NUM_NODES = 4096
NCORES = 8
J = 16           # nodes per variable
VPC = V // NCORES            # 32 variables per core
NPC = VPC * J                # 512 nodes per core
NG = NPC // 128              # 4 groups of 128 nodes (8 vars each)
GV = 128 // J                # 8 variables per group
NE = 2 * C                   # table entries per node (256 cats + 256 sentinel)
CHUNK = 2048                 # batch columns per pipeline iteration
NCHUNK = B // CHUNK          # 2
MM = 512                     # matmul free size (one PSUM bank of fp32)

TRACE = False            # set True (e.g. from test.py) to capture a profile
LAST_RESULT = {}         # exec_time_ns etc. stashed here when TRACE

_MAXW = 1  # this toolchain's walrus encodes at most one sync wait per instruction


def _legalize_waits(nc):
    """Split multi-wait instructions into single-wait NoOp prefixes."""
    for _name, bb in nc.bb_map.items():
        insts = bb.bb.instructions
        new = []
        changed = False
        for ins in insts:
            si = ins.sync_info
            if si is not None and si.on_wait and len(si.on_wait) > _MAXW:
                waits = list(si.on_wait)
                extra, keep = waits[:-_MAXW], waits[-_MAXW:]
                for i, w in enumerate(extra):
                    nop = mybir.InstNoOp(name=f"{ins.name}-sw{i}", ins=[], outs=[])
                    nop.engine = ins.engine
                    nop.sync_info = mybir.SyncInfo(on_wait=[w], on_update=[])
                    new.append(nop)
                ins.sync_info = mybir.SyncInfo(
                    on_wait=keep, on_update=list(si.on_update or [])
                )
                changed = True
            new.append(ins)
        if changed:
            bb.bb.instructions = new


def _build_program():
    nc = bacc.Bacc(
        "TRN2",
        target_bir_lowering=False,
        debug=False,
        num_devices=NCORES,
    )

    tbl = nc.dram_tensor("tbl", [NPC, NE], mybir.dt.float32, kind="ExternalInput")
    d16 = nc.dram_tensor("d16", [128, NG * (B // J)], mybir.dt.int16,
                         kind="ExternalInput")
    alf = nc.dram_tensor("alf", [GV, NG * B], mybir.dt.float32r,
                         kind="ExternalInput")
    wsl = nc.dram_tensor("wsl", [GV, 128], mybir.dt.float32r,
                         kind="ExternalInput")
    out = nc.dram_tensor("out", [NG, 128, B], mybir.dt.float16,
                         kind="ExternalOutput")

    from contextlib import ExitStack

    with tile.TileContext(nc) as tc, ExitStack() as ctx:
        const_pool = ctx.enter_context(tc.tile_pool(name="const", bufs=1))
        g_pool = ctx.enter_context(tc.tile_pool(name="g", bufs=4))
        t_pool = ctx.enter_context(tc.tile_pool(name="t", bufs=4))
        o_pool = ctx.enter_context(tc.tile_pool(name="o", bufs=8))
        ps_pool = ctx.enter_context(tc.tile_pool(name="ps", bufs=2, space="PSUM"))

        t_s = [const_pool.tile([128, NE], mybir.dt.float32, name=f"tsg{g}")
               for g in range(NG)]
        d_s = const_pool.tile([128, NG * (B // J)], mybir.dt.int16)
        a_s = const_pool.tile([GV, NG * B], mybir.dt.float32r)
        w_s = const_pool.tile([GV, 128], mybir.dt.float32r)

        # The first gather needs only t0 + group-0's idx columns; everything
        # else (wsl/alf for the matmuls, t1..t3, remaining idx) can trail.
        # Issue in that order on one engine so the HWDGE/DMA queues match it.
        iw0 = B // J
        nc.sync.dma_start(out=t_s[0][:], in_=tbl[0:128, :])
        nc.sync.dma_start(out=d_s[:, 0:iw0], in_=d16[:, 0:iw0])
        nc.sync.dma_start(out=w_s[:], in_=wsl[:])
        nc.sync.dma_start(out=a_s[:, 0:CHUNK], in_=alf[:, 0:CHUNK])
        nc.sync.dma_start(out=a_s[:, CHUNK:NG * B], in_=alf[:, CHUNK:NG * B])
        nc.sync.dma_start(out=d_s[:, iw0:NG * iw0], in_=d16[:, iw0:NG * iw0])
        for g in range(1, NG):
            nc.sync.dma_start(out=t_s[g][:], in_=tbl[g * 128:(g + 1) * 128, :])

        # Warm the ACT Ln table (1283ns load) off the critical path.
        warm = const_pool.tile([GV, 128], mybir.dt.float32)
        nc.scalar.activation(
            out=warm[:], in_=w_s[:],
            func=mybir.ActivationFunctionType.Ln, bias=1.0)

        # (group, col offset, cols): full chunks, with the tail tapered so the
        # final DVE->ACT->DMA drain after the last gather is short.
        sched = []
        for g in range(NG - 1):
            for h in range(NCHUNK):
                sched.append((g, h * CHUNK, CHUNK))
        c0 = 0
        for cl in (1024, 1024, 1024, 512, 256, 256):
            sched.append((NG - 1, c0, cl))
            c0 += cl

        iw = B // J          # idx free columns per group (256)
        for g, c0, cl in sched:
            f0 = g * iw + c0 // J              # idx tile free offset
            G = g_pool.tile([128, cl], mybir.dt.float32, tag="G")
            nc.gpsimd.ap_gather(
                out_ap=G[:], in_ap=t_s[g][:],
                idxs_ap=d_s[:, f0:f0 + cl // J],
                channels=128, num_elems=NE, d=1, num_idxs=cl)

            A = ps_pool.tile([128, cl], mybir.dt.float32, tag="A")
            for m in range(-(-cl // MM)):
                m0, m1 = m * MM, min((m + 1) * MM, cl)
                nc.tensor.matmul(
                    out=A[:, m0:m1],
                    lhsT=w_s[:],
                    rhs=a_s[:, g * B + c0 + m0:g * B + c0 + m1],
                    start=True, stop=True)

            T = t_pool.tile([128, cl], mybir.dt.float32, tag="T")
            nc.vector.tensor_tensor(
                out=T[:], in0=G[:], in1=A[:], op=mybir.AluOpType.mult)

            O = o_pool.tile([128, cl], mybir.dt.float16, tag="O")
            nc.scalar.activation(
                out=O[:], in_=T[:],
                func=mybir.ActivationFunctionType.Ln, bias=1.0)
            nc.sync.dma_start(out=out[g, :, c0:c0 + cl], in_=O[:])

    nc.compile()
    _legalize_waits(nc)
    return nc


_prog_cache = {}


def _get_program(key=None):
    if "p" not in _prog_cache:
        _prog_cache["p"] = _build_program()
    return _prog_cache["p"]


def kernel(data, vids, psids, params, missing_mask, alphas):
    data = np.asarray(data).astype(np.int32, copy=False)
    vids = np.asarray(vids).astype(np.int64, copy=False)
    psids = np.asarray(psids).astype(np.int64, copy=False)
    params = np.asarray(params).astype(np.float32, copy=False)
    missing = np.asarray(missing_mask).astype(bool, copy=False)
    alphas = np.asarray(alphas).astype(np.float32, copy=False)

    num_nodes = vids.shape[0]
    assert num_nodes == NUM_NODES and data.shape == (V, B)

    # node -> variable map; each variable must own J consecutive nodes
    vb = vids.reshape(NUM_NODES // J, J)
    assert (vb == vb[:, :1]).all(), "nodes of a 16-block must share a variable"
    blk_vid = vb[:, 0]                                   # [256] variable per block

    # full per-node category tables, storing q - 1
    gi = psids[:, None] + np.arange(C, dtype=np.int64)[None, :]
    tfull = params[gi].astype(np.float32) - np.float32(1.0)   # [4096, 256]

    # combined gather index (data + 256*missing), int16
    dcomb = (data + (missing.astype(np.int32) << 8)).astype(np.int16)  # [V, B]

    in_maps = []
    for ci in range(NCORES):
        n0 = ci * NPC
        vlist = blk_vid[ci * VPC:(ci + 1) * VPC]          # [32] vars of this core

        tblh = np.zeros((NPC, NE), dtype=np.float32)
        tblh[:, :C] = tfull[n0:n0 + NPC]

        # wrapped idx layout: partition 16j+s, free 256g+f  <-  dc[8g+j, 16f+s]
        dc = dcomb[vlist]                                 # [32, B]
        d16h = (dc.reshape(NG, GV, B // J, J)
                  .transpose(1, 3, 0, 2)
                  .reshape(128, NG * (B // J)))

        # alphas: row j, col g*B+b  <-  alphas[vlist[8g+j], b]
        alh = (alphas[vlist].reshape(NG, GV, B)
                            .transpose(1, 0, 2)
                            .reshape(GV, NG * B))

        wsh = np.repeat(np.eye(GV, dtype=np.float32), J, axis=1)  # [8, 128]

        in_maps.append(dict(
            tbl=tblh,
            d16=np.ascontiguousarray(d16h),
            alf=np.ascontiguousarray(alh),
            wsl=wsh,
        ))

    nc = _get_program()
    res = run_bass_kernel_spmd(nc, in_maps, list(range(NCORES)), trace=TRACE)
    if TRACE:
        LAST_RESULT["exec_time_ns"] = getattr(res, "exec_time_ns", None)
        LAST_RESULT["mean_exec_time_ns"] = getattr(res, "mean_exec_time_ns", None)
        LAST_RESULT["profile_json"] = getattr(res, "profile_json", None)

    out_full = np.empty((NUM_NODES, B), dtype=np.float32)
    for ci in range(NCORES):
        o = np.asarray(res.results[ci]["out"])            # [4, 128, 4096] fp16
        out_full[ci * NPC:(ci + 1) * NPC] = o.reshape(NPC, B).astype(np.float32)
    return out_full
